# revision 43
# baseline (speedup 1.0000x reference)
"""BitNet FFN Trainium2 kernel (8-core SPMD, data-parallel over tokens).

Math (forward values of the STE reference):
  wq(w)  = clip(round(w/s), -1, 1) * s,  s = mean(|w|) + EPS        (ternary)
  xq(x)  = round(x/sx) * sx,  sx = max(absmax_row(x), EPS)/127      (int8 range)
  gate = sigmoid(xq @ wq_g.T); up = xq @ wq_u.T; h = gate*up
  out  = hq(h) @ wq_d.T

Every matmul runs in bf16 with fp32 PSUM accumulation on exact integers
(|int| <= 127 activations, ternary weights, partial sums < 2^24), so the
integer matmuls are exact; all scales are folded in fp32 outside the
matmuls.  Tokens are sharded 8 ways (1024/core); each core streams the
full weights once.  The only collective is a 16-byte AllReduce for the
three global weight-scale sums.

Phase-1 produces h' directly in [ff, tok] layout (stationary operand =
transposed ternary weight block, moving operand = transposed quantized
activations), so the phase-3 contraction input hqt needs no transpose:
h' is spilled to DRAM fp32 and read back contiguously.  Per-token scales
live as broadcast rows [P, T].
"""

import sys

sys.path.insert(0, "/opt/trn_rl_repo")

import numpy as np

import concourse.tile as tile
from concourse import bacc, mybir, bass_isa

F32 = mybir.dt.float32
BF16 = mybir.dt.bfloat16
ADD = mybir.AluOpType.add
SUB = mybir.AluOpType.subtract
MULT = mybir.AluOpType.mult
MAX = mybir.AluOpType.max
ABSMAX = mybir.AluOpType.abs_max
BYPASS = mybir.AluOpType.bypass
AXX = mybir.AxisListType.X
AFT = mybir.ActivationFunctionType
RED = bass_isa.ReduceOp

EPS = 1e-5
CR = 12582912.0  # 1.5*2^23: fp32 RNE round-to-integer magic constant
ALPHA = 1.0986122886681098  # atanh(0.5)/0.5 : tanh(ALPHA*0.5) == 0.5
P = 128


def build_program(T, DM, FF, ncores, ff_sh, dm_sh):
    """Build the per-core SPMD program.

    T: tokens per core; DM: d_model; FF: d_ff; ff_sh/dm_sh: rows of the
    per-core weight-scale shards (w_gate/w_up shard rows, w_down shard rows).
    """
    assert T % P == 0 and DM % P == 0 and FF % P == 0
    MT = T // P              # token tiles
    KD = DM // P             # d_model k-blocks
    NFB = FF // P            # ff blocks (phase-1 output blocks / phase-3 k)
    MD = DM // P             # output dm blocks
    TCH = min(512, T)        # token chunk (psum free dim)
    NTC = T // TCH           # token chunks
    KW3 = min(2048, FF)      # phase-3 wd piece width
    NW3 = FF // KW3          # wd pieces per md
    KB3 = KW3 // P           # k-blocks per wd piece

    nc = bacc.Bacc(
        "TRN2",
        target_bir_lowering=False,
        debug=False,
        enable_asserts=False,
        num_devices=ncores,
    )

    x_d = nc.dram_tensor("x", [T, DM], F32, kind="ExternalInput")
    wg_d = nc.dram_tensor("wg", [FF, DM], F32, kind="ExternalInput")
    wu_d = nc.dram_tensor("wu", [FF, DM], F32, kind="ExternalInput")
    wd_d = nc.dram_tensor("wd", [DM, FF], F32, kind="ExternalInput")
    wgs_d = nc.dram_tensor("wg_sh", [ff_sh, DM], F32, kind="ExternalInput")
    wus_d = nc.dram_tensor("wu_sh", [ff_sh, DM], F32, kind="ExternalInput")
    wds_d = nc.dram_tensor("wd_sh", [dm_sh, FF], F32, kind="ExternalInput")
    out_d = nc.dram_tensor("out_t", [DM, T], F32, kind="ExternalOutput")

    NW = float(FF * DM)  # elements per weight matrix (all three equal)

    with tile.TileContext(nc, num_cores=ncores) as tc:
        import contextlib

        with contextlib.ExitStack() as outer:
            dram = outer.enter_context(tc.tile_pool(name="dram", bufs=1, space="DRAM"))
            psum = outer.enter_context(tc.tile_pool(name="psum", bufs=8, space="PSUM"))
            tiny = outer.enter_context(tc.tile_pool(name="tiny", bufs=1))
            # rph/shd broadcast rows survive into phase 3
            bc2_p = outer.enter_context(tc.tile_pool(name="bc2", bufs=2))

            hp_d = dram.tile([NFB, NTC, P, TCH], F32)  # h' blocked fp32
            sx_d = dram.tile([1, T], F32)              # per-token x scale row
            rph_d = dram.tile([1, T], F32)             # per-token h quant scale
            shd_d = dram.tile([1, T], F32)             # per-token out scale
            cc_in = dram.tile([1, 4], F32)
            cc_out = dram.tile([1, 4], F32)
            cc0_in = dram.tile([1, 1], F32)
            cc0_out = dram.tile([1, 1], F32)
            # pay the cross-core sync barrier up front, overlapped with S0
            nc.gpsimd.collective_compute(
                "AllReduce",
                ADD,
                replica_groups=[list(range(ncores))],
                ins=[cc0_in[:].opt()],
                outs=[cc0_out[:].opt()],
            )

            sb_scales = tiny.tile([P, 8], F32)   # bcast: bg,bu,bd,-,swg,swu,swd,-
            sx_all = tiny.tile([P, MT], F32)     # per-token x scale (col=tile)
            rx_all = tiny.tile([P, MT], F32)
            sxu_all = tiny.tile([P, MT], F32)    # sx*swu columns
            ones_col = tiny.tile([P, 1], F32)
            nc.vector.memset(ones_col, 1.0)
            ones_row = tiny.tile([1, P], F32)
            nc.vector.memset(ones_row, 1.0)
            ident = tiny.tile([P, P], F32)       # for PE transpose
            nc.vector.memset(ident, 1.0)
            nc.gpsimd.affine_select(
                out=ident, in_=ident, pattern=[[1, P]],
                compare_op=mybir.AluOpType.is_equal, fill=0.0,
                base=0, channel_multiplier=-1,
            )

            def pe_broadcast_row(row, dst):
                """dst[P, T] = broadcast of row [1, T] to all partitions."""
                for c0 in range(0, T, TCH):
                    psb = psum.tile([P, TCH], F32, name="ps_main")
                    nc.tensor.matmul(
                        psb, ones_row, row[:, c0 : c0 + TCH],
                        start=True, stop=True,
                    )
                    nc.vector.tensor_copy(dst[:, c0 : c0 + TCH], psb)

            # ---------------- S0: global weight scales ----------------
            ph1 = contextlib.ExitStack()
            wraw_p = ph1.enter_context(tc.tile_pool(name="wraw", bufs=4))
            with tc.tile_pool(name="s0", bufs=8) as s0p, tc.tile_pool(
                name="s0t", bufs=8
            ) as s0t:
                acc3 = tiny.tile([P, 4], F32)
                nc.vector.memset(acc3, 0.0)
                # (src, acc col, rows, cols, piece width, load engine, abs engine)
                shard_specs = [
                    (wgs_d, 0, ff_sh, DM, min(2048, DM), nc.sync),
                    (wus_d, 1, ff_sh, DM, min(2048, DM), nc.scalar),
                    (wds_d, 2, dm_sh, FF, min(2048, FF), nc.scalar),
                ]
                idx = 0
                for src, col, rows, cols, pw, ldeng in shard_specs:
                    for r0 in range(0, rows, P):
                        pr = min(P, rows - r0)
                        for c0 in range(0, cols, pw):
                            t_in = s0p.tile([P, pw], F32, name="s0raw")
                            ldeng.dma_start(
                                t_in[:pr], src[r0 : r0 + pr, c0 : c0 + pw]
                            )
                            t_sum = s0t.tile([P, 1], F32, name="s0sum")
                            if idx % 2 == 0:
                                t_abs = s0p.tile([P, pw], F32, name="s0abs")
                                nc.scalar.activation(
                                    out=t_abs[:pr],
                                    in_=t_in[:pr],
                                    func=AFT.Abs,
                                    accum_out=t_sum[:pr],
                                )
                            else:
                                nc.vector.tensor_reduce(
                                    t_sum[:pr], t_in[:pr], axis=AXX, op=ADD,
                                    apply_absolute_value=True,
                                )
                            nc.vector.tensor_tensor(
                                out=acc3[:pr, col : col + 1],
                                in0=acc3[:pr, col : col + 1],
                                in1=t_sum[:pr],
                                op=ADD,
                            )
                            idx += 1
                # prefetch the first weight strips while the AllReduce
                # (queued behind on gpsimd) waits for the sums
                raw_pre = {}
                for b in range(2):
                    for wsrc, kk, ldeng in (
                        (wg_d, "g", nc.gpsimd), (wu_d, "u", nc.sync)
                    ):
                        rt = wraw_p.tile([P, DM], F32, name="wraw")
                        ldeng.dma_start(rt, wsrc[b * P : (b + 1) * P, :])
                        raw_pre[(b, kk)] = rt
                # cross-partition sum + 16B AllReduce for global sums
                ps_s = psum.tile([P, TCH], F32, name="ps_main")
                nc.tensor.matmul(
                    ps_s[:4, :1], acc3[:, :4], ones_col, start=True, stop=True
                )
                sb_s = s0t.tile([4, 1], F32, name="sb_s")
                nc.vector.tensor_copy(sb_s, ps_s[:4, :1])
                nc.sync.dma_start(cc_in[0, :4], sb_s[:, 0])
                nc.gpsimd.collective_compute(
                    "AllReduce",
                    ADD,
                    replica_groups=[list(range(ncores))],
                    ins=[cc_in[:].opt()],
                    outs=[cc_out[:].opt()],
                )

            # ---------------- phase 1: x-quant + gate/up -> h' [ff,tok] ----
            with ph1:
                xqt_p = ph1.enter_context(tc.tile_pool(name="xqt", bufs=1))
                xqt = xqt_p.tile([P, KD, T], BF16)  # XqT: [dm-part, k, token]
                acc_p = ph1.enter_context(tc.tile_pool(name="accp", bufs=1))
                acc_hi = acc_p.tile([P, T], F32)    # h' max partials (per tok)
                acc_lo = acc_p.tile([P, T], F32)    # h' min partials (per tok)
                nc.vector.memset(acc_hi, 0.0)
                nc.vector.memset(acc_lo, 0.0)
                bc_p = ph1.enter_context(tc.tile_pool(name="bc", bufs=4))

                # x quantization (per token-tile; loads on the sync queue —
                # the gpsimd queue is head-of-line blocked by the AllReduce
                # trigger until the S0 sums are ready)
                with tc.tile_pool(name="xw", bufs=4) as xw_p:
                    for m in range(MT):
                        xt = xw_p.tile([P, DM], F32, name="xt")
                        nc.sync.dma_start(xt, x_d[m * P : (m + 1) * P, :])
                        amax = xw_p.tile([P, 1], F32, name="amax")
                        nc.vector.tensor_reduce(
                            amax, xt, axis=AXX, op=MAX, apply_absolute_value=True
                        )
                        nc.vector.tensor_scalar(
                            out=sx_all[:, m : m + 1], in0=amax, scalar1=EPS,
                            scalar2=1.0 / 127.0, op0=MAX, op1=MULT,
                        )
                        nc.vector.reciprocal(
                            rx_all[:, m : m + 1], sx_all[:, m : m + 1]
                        )
                        xr = xw_p.tile([P, DM], F32, name="xr")
                        nc.vector.tensor_scalar(
                            out=xr, in0=xt, scalar1=rx_all[:, m : m + 1], scalar2=CR,
                            op0=MULT, op1=ADD,
                        )
                        xq = xw_p.tile([P, DM], BF16, name="xq")
                        nc.vector.tensor_scalar(
                            out=xq, in0=xr, scalar1=CR, scalar2=None,
                            op0=SUB, op1=BYPASS,
                        )
                        nc.sync.dma_start_transpose(
                            xqt[:, :, m * P : (m + 1) * P], xq
                        )
                        # stash sx column into the [1,T] row for broadcasting
                        nc.sync.dma_start(
                            sx_d[0, m * P : (m + 1) * P], sx_all[:, m : m + 1]
                        )

                # ---- S0 tail (gated on the AllReduce): emitted after
                # x-quant so the AR wait doesn't head-of-line block the
                # vector queue during startup
                sums_row = bc_p.tile([1, T], F32, name="bc_row")
                nc.sync.dma_start(sums_row[:, 0:4], cc_out[:])
                sw_row = bc_p.tile([1, T], F32, name="bc_row")
                nc.vector.tensor_scalar(
                    out=sw_row[:, 0:4], in0=sums_row[:, 0:4], scalar1=1.0 / NW,
                    scalar2=EPS, op0=MULT, op1=ADD,
                )
                nc.vector.reciprocal(sw_row[:, 4:8], sw_row[:, 0:4])
                row8 = bc_p.tile([1, T], F32, name="bc_row")
                nc.vector.tensor_scalar(
                    out=row8[:, 0:4], in0=sw_row[:, 4:8], scalar1=ALPHA,
                    scalar2=None, op0=MULT, op1=BYPASS,
                )
                nc.vector.tensor_copy(row8[:, 4:8], sw_row[:, 0:4])
                ps_b = psum.tile([P, TCH], F32, name="ps_main")
                nc.tensor.matmul(
                    ps_b[:, :8], ones_row, row8[:, 0:8], start=True, stop=True
                )
                nc.vector.tensor_copy(sb_scales, ps_b[:, :8])

                # per-token scale broadcast rows [P, T]
                sx_row = bc_p.tile([1, T], F32, name="bc_row")
                nc.sync.dma_start(sx_row, sx_d[:])
                sx_bc = bc_p.tile([P, T], F32, name="bc")
                pe_broadcast_row(sx_row, sx_bc)
                sxg_bc = bc_p.tile([P, T], F32, name="bc")
                nc.vector.tensor_scalar(
                    out=sxg_bc, in0=sx_bc, scalar1=sb_scales[:, 4:5],
                    scalar2=None, op0=MULT, op1=BYPASS,
                )
                sxu_bc = bc_p.tile([P, T], F32, name="bc")
                nc.vector.tensor_scalar(
                    out=sxu_bc, in0=sx_bc, scalar1=sb_scales[:, 5:6],
                    scalar2=None, op0=MULT, op1=BYPASS,
                )
                nc.vector.tensor_scalar(
                    out=sxu_all, in0=sx_all, scalar1=sb_scales[:, 5:6],
                    scalar2=None, op0=MULT, op1=BYPASS,
                )

                wtern_p = ph1.enter_context(tc.tile_pool(name="wtern", bufs=4))
                wchunk_p = ph1.enter_context(tc.tile_pool(name="wchunk", bufs=15))
                gt_p = ph1.enter_context(tc.tile_pool(name="gtp", bufs=4))
                hpr_p = ph1.enter_context(tc.tile_pool(name="hpr", bufs=4))

                def produce_chunk(wsrc, beta_col, b, teng, raw=None,
                                  ldeng=nc.gpsimd):
                    if raw is None:
                        raw = wraw_p.tile([P, DM], F32, name="wraw")
                        ldeng.dma_start(raw, wsrc[b * P : (b + 1) * P, :])
                    nc.scalar.activation(
                        out=raw, in_=raw, func=AFT.Tanh,
                        scale=sb_scales[:, beta_col : beta_col + 1],
                    )
                    tern = wtern_p.tile([P, DM], BF16, name="wtern")
                    nc.vector.tensor_scalar(
                        out=tern, in0=raw, scalar1=CR, scalar2=CR,
                        op0=ADD, op1=SUB,
                    )
                    ch = wchunk_p.tile([P, KD, P], BF16, name="wchunk")
                    teng.dma_start_transpose(ch, tern)
                    return ch

                for b in range(NFB):
                    chg = produce_chunk(
                        wg_d, 0, b, nc.sync, raw=raw_pre.get((b, "g")),
                        ldeng=nc.gpsimd,
                    )
                    chu = produce_chunk(
                        wu_d, 1, b, nc.sync, raw=raw_pre.get((b, "u")),
                        ldeng=nc.sync,
                    )
                    psg = [
                        psum.tile([P, TCH], F32, name="ps_main")
                        for _ in range(NTC)
                    ]
                    psu = [
                        psum.tile([P, TCH], F32, name="ps_main")
                        for _ in range(NTC)
                    ]
                    for k in range(KD):
                        st, sp = (k == 0), (k == KD - 1)
                        for t in range(NTC):
                            nc.tensor.matmul(
                                psg[t], chg[:, k, :],
                                xqt[:, k, t * TCH : (t + 1) * TCH],
                                start=st, stop=sp,
                            )
                        for t in range(NTC):
                            nc.tensor.matmul(
                                psu[t], chu[:, k, :],
                                xqt[:, k, t * TCH : (t + 1) * TCH],
                                start=st, stop=sp,
                            )
                    for t in range(NTC):
                        sl = slice(t * TCH, (t + 1) * TCH)
                        gt = gt_p.tile([P, TCH], F32, name="gt")
                        nc.vector.tensor_tensor(
                            out=gt, in0=psg[t], in1=sxg_bc[:, sl], op=MULT
                        )
                        nc.scalar.activation(out=gt, in_=gt, func=AFT.Sigmoid)
                        hp = hpr_p.tile([P, TCH], F32, name="hp")
                        nc.vector.tensor_tensor(
                            out=hp, in0=gt, in1=psu[t], op=MULT
                        )
                        nc.vector.tensor_tensor(
                            out=acc_hi[:, sl], in0=acc_hi[:, sl], in1=hp,
                            op=MAX,
                        )
                        nc.vector.tensor_tensor(
                            out=acc_lo[:, sl], in0=acc_lo[:, sl], in1=hp,
                            op=mybir.AluOpType.min,
                        )
                        nc.scalar.dma_start(hp_d[b, t], hp)

                # ---- h quantization scales ----
                # per-token absmax: PE-transpose the [ff-pos, tok] partials to
                # token-partition columns, reduce along free axis, then do the
                # scale math as columns [P, 1] per token tile (baseline form).
                with tc.tile_pool(name="hscl", bufs=8) as hs_p:
                    for m in range(MT):
                        msl = slice(m * P, (m + 1) * P)
                        pthi = psum.tile([P, P], F32, name="ps_main")
                        nc.tensor.transpose(pthi, acc_hi[:, msl], ident)
                        ptlo = psum.tile([P, P], F32, name="ps_main")
                        nc.tensor.transpose(ptlo, acc_lo[:, msl], ident)
                        chi = hs_p.tile([P, 1], F32, name="chi")
                        nc.vector.tensor_reduce(
                            chi, pthi, axis=AXX, op=MAX,
                            apply_absolute_value=True,
                        )
                        clo = hs_p.tile([P, 1], F32, name="clo")
                        nc.vector.tensor_reduce(
                            clo, ptlo, axis=AXX, op=MAX,
                            apply_absolute_value=True,
                        )
                        habs_c = hs_p.tile([P, 1], F32, name="habs_c")
                        nc.vector.tensor_tensor(
                            out=habs_c, in0=chi, in1=clo, op=MAX
                        )
                        sh_c = hs_p.tile([P, 1], F32, name="sh_c")
                        nc.vector.tensor_tensor(
                            out=sh_c, in0=habs_c, in1=sxu_all[:, m : m + 1],
                            op=MULT,
                        )
                        nc.vector.tensor_scalar(
                            out=sh_c, in0=sh_c, scalar1=EPS,
                            scalar2=1.0 / 127.0, op0=MAX, op1=MULT,
                        )
                        rec_c = hs_p.tile([P, 1], F32, name="rec_c")
                        nc.vector.reciprocal(rec_c, sh_c)
                        rph_c = hs_p.tile([P, 1], F32, name="rph_c")
                        nc.vector.tensor_tensor(
                            out=rph_c, in0=rec_c, in1=sxu_all[:, m : m + 1],
                            op=MULT,
                        )
                        shd_c = hs_p.tile([P, 1], F32, name="shd_c")
                        nc.vector.tensor_scalar(
                            out=shd_c, in0=sh_c, scalar1=sb_scales[:, 6:7],
                            scalar2=None, op0=MULT, op1=BYPASS,
                        )
                        nc.sync.dma_start(rph_d[0, msl], rph_c[:, 0:1])
                        nc.sync.dma_start(shd_d[0, msl], shd_c[:, 0:1])

                    rph_row = bc_p.tile([1, T], F32, name="bc_row")
                    nc.sync.dma_start(rph_row, rph_d[:])
                    rph_bc = bc2_p.tile([P, T], F32, name="bc2")
                    pe_broadcast_row(rph_row, rph_bc)
                    shd_row = bc_p.tile([1, T], F32, name="bc_row")
                    nc.sync.dma_start(shd_row, shd_d[:])
                    shd_bc = bc2_p.tile([P, T], F32, name="bc2")
                    pe_broadcast_row(shd_row, shd_bc)

            # ---------------- phase 3: quantize h' + down projection -------
            with contextlib.ExitStack() as ph3:
                hqt_p = ph3.enter_context(tc.tile_pool(name="hqt", bufs=1))
                hqt = hqt_p.tile([P, NFB, T], BF16)  # [ff-in-blk, ff-blk, tok]
                stage_p = ph3.enter_context(tc.tile_pool(name="stage", bufs=2))
                wdr_p = ph3.enter_context(tc.tile_pool(name="wdr", bufs=2))
                wdtern_p = ph3.enter_context(tc.tile_pool(name="wdtn", bufs=2))
                wdt_p = ph3.enter_context(tc.tile_pool(name="wdtg", bufs=2 * NW3))
                fin_p = ph3.enter_context(tc.tile_pool(name="finp", bufs=2))

                def quantize_block(b):
                    stage = stage_p.tile([P, T], F32, name="stage")
                    for t in range(NTC):
                        nc.gpsimd.dma_start(
                            stage[:, t * TCH : (t + 1) * TCH], hp_d[b, t]
                        )
                    stage2 = stage_p.tile([P, T], F32, name="stage")
                    nc.vector.tensor_tensor(
                        out=stage2, in0=stage, in1=rph_bc, op=MULT
                    )
                    nc.vector.tensor_scalar(
                        out=hqt[:, b, :], in0=stage2, scalar1=CR, scalar2=CR,
                        op0=ADD, op1=SUB,
                    )

                def produce_wd(md):
                    pieces = []
                    for w in range(NW3):
                        raw = wdr_p.tile([P, KW3], F32, name="wdraw")
                        nc.gpsimd.dma_start(
                            raw,
                            wd_d[md * P : (md + 1) * P, w * KW3 : (w + 1) * KW3],
                        )
                        nc.scalar.activation(
                            out=raw, in_=raw, func=AFT.Tanh,
                            scale=sb_scales[:, 2:3],
                        )
                        ternd = wdtern_p.tile([P, KW3], BF16, name="wdtern")
                        nc.vector.tensor_scalar(
                            out=ternd, in0=raw, scalar1=CR, scalar2=CR,
                            op0=ADD, op1=SUB,
                        )
                        wdtg = wdt_p.tile([P, KB3, P], BF16, name="wdtg")
                        nc.sync.dma_start_transpose(wdtg, ternd)
                        pieces.append(wdtg)
                    return pieces

                # Emit the first mds' weight pipelines ahead of the hqt fill
                # so their vector/scalar work isn't queued behind it; all
                # hqt writers must be emitted before any consuming matmul.
                NAHEAD = min(2, MD)
                wd_pieces = {md: produce_wd(md) for md in range(NAHEAD)}
                for b in range(NFB):
                    quantize_block(b)
                def emit_fin(md, pss):
                    for t in range(NTC):
                        sl = slice(t * TCH, (t + 1) * TCH)
                        ot = fin_p.tile([P, TCH], F32, name="ot")
                        nc.vector.tensor_tensor(
                            out=ot, in0=pss[t], in1=shd_bc[:, sl], op=MULT
                        )
                        nc.scalar.dma_start(
                            out_d[md * P : (md + 1) * P, sl], ot
                        )

                pending = []
                for md in range(MD):
                    pieces = wd_pieces.pop(md) if md in wd_pieces else produce_wd(md)
                    pss = [
                        psum.tile([P, TCH], F32, name="ps_main")
                        for _ in range(NTC)
                    ]
                    for k in range(NFB):
                        lhsT = pieces[k // KB3][:, k % KB3, :]
                        st, sp = (k == 0), (k == NFB - 1)
                        for t in range(NTC):
                            nc.tensor.matmul(
                                pss[t], lhsT,
                                hqt[:, k, t * TCH : (t + 1) * TCH],
                                start=st, stop=sp,
                            )
                    pending.append((md, pss))
                    if len(pending) >= 3:
                        emit_fin(*pending.pop(0))
                for it in pending:
                    emit_fin(*it)

    nc.compile()
    return nc


_CACHE = {}
TRACE = False  # set True (e.g. from test.py) to capture an NTFF profile
LAST_RESULTS = None


def _get_program(T, DM, FF, ncores, ff_sh, dm_sh):
    key = (T, DM, FF, ncores, ff_sh, dm_sh)
    if key not in _CACHE:
        _CACHE[key] = build_program(T, DM, FF, ncores, ff_sh, dm_sh)
    return _CACHE[key]


def kernel(x, w_gate, w_up, w_down):
    from concourse.bass_utils import run_bass_kernel_spmd

    x = np.asarray(x, dtype=np.float32)
    w_gate = np.ascontiguousarray(np.asarray(w_gate, dtype=np.float32))
    w_up = np.ascontiguousarray(np.asarray(w_up, dtype=np.float32))
    w_down = np.ascontiguousarray(np.asarray(w_down, dtype=np.float32))

    B, S, DM = x.shape
    FF = w_gate.shape[0]
    NCORES = 8
    NTOK = B * S
    T = NTOK // NCORES
    ff_sh = FF // NCORES
    dm_sh = DM // NCORES

    xf = np.ascontiguousarray(x.reshape(NTOK, DM))
    nc = _get_program(T, DM, FF, NCORES, ff_sh, dm_sh)

    in_maps = []
    for c in range(NCORES):
        in_maps.append(
            {
                "x": np.ascontiguousarray(xf[c * T : (c + 1) * T]),
                "wg": w_gate,
                "wu": w_up,
                "wd": w_down,
                "wg_sh": np.ascontiguousarray(w_gate[c * ff_sh : (c + 1) * ff_sh]),
                "wu_sh": np.ascontiguousarray(w_up[c * ff_sh : (c + 1) * ff_sh]),
                "wd_sh": np.ascontiguousarray(w_down[c * dm_sh : (c + 1) * dm_sh]),
            }
        )

    res = run_bass_kernel_spmd(
        nc, in_maps, core_ids=list(range(NCORES)), trace=TRACE
    )
    global LAST_RESULTS
    LAST_RESULTS = res
    out = np.empty((NTOK, DM), dtype=np.float32)
    for c in range(NCORES):
        out[c * T : (c + 1) * T] = res.results[c]["out_t"].T
    return out.reshape(B, S, DM)


# revision 44
# speedup vs baseline: 1.0139x; 1.0139x over previous
"""BitNet FFN Trainium2 kernel (8-core SPMD, data-parallel over tokens).

Math (forward values of the STE reference):
  wq(w)  = clip(round(w/s), -1, 1) * s,  s = mean(|w|) + EPS        (ternary)
  xq(x)  = round(x/sx) * sx,  sx = max(absmax_row(x), EPS)/127      (int8 range)
  gate = sigmoid(xq @ wq_g.T); up = xq @ wq_u.T; h = gate*up
  out  = hq(h) @ wq_d.T

Every matmul runs in bf16 with fp32 PSUM accumulation on exact integers
(|int| <= 127 activations, ternary weights, partial sums < 2^24), so the
integer matmuls are exact; all scales are folded in fp32 outside the
matmuls.  Tokens are sharded 8 ways (1024/core); each core streams the
full weights once.  The only collective is a 16-byte AllReduce for the
three global weight-scale sums.

Phase-1 produces h' directly in [ff, tok] layout (stationary operand =
transposed ternary weight block, moving operand = transposed quantized
activations), so the phase-3 contraction input hqt needs no transpose:
h' is spilled to DRAM fp32 and read back contiguously.  Per-token scales
live as broadcast rows [P, T].
"""

import sys

sys.path.insert(0, "/opt/trn_rl_repo")

import numpy as np

import concourse.tile as tile
from concourse import bacc, mybir, bass_isa

F32 = mybir.dt.float32
BF16 = mybir.dt.bfloat16
ADD = mybir.AluOpType.add
SUB = mybir.AluOpType.subtract
MULT = mybir.AluOpType.mult
MAX = mybir.AluOpType.max
ABSMAX = mybir.AluOpType.abs_max
BYPASS = mybir.AluOpType.bypass
AXX = mybir.AxisListType.X
AFT = mybir.ActivationFunctionType
RED = bass_isa.ReduceOp

EPS = 1e-5
CR = 12582912.0  # 1.5*2^23: fp32 RNE round-to-integer magic constant
ALPHA = 1.0986122886681098  # atanh(0.5)/0.5 : tanh(ALPHA*0.5) == 0.5
P = 128


def build_program(T, DM, FF, ncores, ff_sh, dm_sh):
    """Build the per-core SPMD program.

    T: tokens per core; DM: d_model; FF: d_ff; ff_sh/dm_sh: rows of the
    per-core weight-scale shards (w_gate/w_up shard rows, w_down shard rows).
    """
    assert T % P == 0 and DM % P == 0 and FF % P == 0
    MT = T // P              # token tiles
    KD = DM // P             # d_model k-blocks
    NFB = FF // P            # ff blocks (phase-1 output blocks / phase-3 k)
    MD = DM // P             # output dm blocks
    TCH = min(512, T)        # token chunk (psum free dim)
    NTC = T // TCH           # token chunks
    KW3 = min(2048, FF)      # phase-3 wd piece width
    NW3 = FF // KW3          # wd pieces per md
    KB3 = KW3 // P           # k-blocks per wd piece

    nc = bacc.Bacc(
        "TRN2",
        target_bir_lowering=False,
        debug=False,
        enable_asserts=False,
        num_devices=ncores,
    )

    x_d = nc.dram_tensor("x", [T, DM], F32, kind="ExternalInput")
    wg_d = nc.dram_tensor("wg", [FF, DM], F32, kind="ExternalInput")
    wu_d = nc.dram_tensor("wu", [FF, DM], F32, kind="ExternalInput")
    wd_d = nc.dram_tensor("wd", [DM, FF], F32, kind="ExternalInput")
    wgs_d = nc.dram_tensor("wg_sh", [ff_sh, DM], F32, kind="ExternalInput")
    wus_d = nc.dram_tensor("wu_sh", [ff_sh, DM], F32, kind="ExternalInput")
    wds_d = nc.dram_tensor("wd_sh", [dm_sh, FF], F32, kind="ExternalInput")
    out_d = nc.dram_tensor("out_t", [DM, T], F32, kind="ExternalOutput")

    NW = float(FF * DM)  # elements per weight matrix (all three equal)

    with tile.TileContext(nc, num_cores=ncores) as tc:
        import contextlib

        with contextlib.ExitStack() as outer:
            dram = outer.enter_context(tc.tile_pool(name="dram", bufs=1, space="DRAM"))
            psum = outer.enter_context(tc.tile_pool(name="psum", bufs=8, space="PSUM"))
            tiny = outer.enter_context(tc.tile_pool(name="tiny", bufs=1))
            # rph/shd broadcast rows survive into phase 3
            bc2_p = outer.enter_context(tc.tile_pool(name="bc2", bufs=2))

            hp_d = dram.tile([NFB, NTC, P, TCH], F32)  # h' blocked fp32
            sx_d = dram.tile([1, T], F32)              # per-token x scale row
            rph_d = dram.tile([1, T], F32)             # per-token h quant scale
            shd_d = dram.tile([1, T], F32)             # per-token out scale
            cc_in = dram.tile([1, 4], F32)
            cc_out = dram.tile([1, 4], F32)
            cc0_in = dram.tile([1, 1], F32)
            cc0_out = dram.tile([1, 1], F32)
            # pay the cross-core sync barrier up front, overlapped with S0
            nc.gpsimd.collective_compute(
                "AllReduce",
                ADD,
                replica_groups=[list(range(ncores))],
                ins=[cc0_in[:].opt()],
                outs=[cc0_out[:].opt()],
            )

            sb_scales = tiny.tile([P, 8], F32)   # bcast: bg,bu,bd,-,swg,swu,swd,-
            sx_all = tiny.tile([P, MT], F32)     # per-token x scale (col=tile)
            rx_all = tiny.tile([P, MT], F32)
            sxu_all = tiny.tile([P, MT], F32)    # sx*swu columns
            ones_col = tiny.tile([P, 1], F32)
            nc.vector.memset(ones_col, 1.0)
            ones_row = tiny.tile([1, P], F32)
            nc.vector.memset(ones_row, 1.0)
            ident = tiny.tile([P, P], F32)       # for PE transpose
            nc.vector.memset(ident, 1.0)
            nc.gpsimd.affine_select(
                out=ident, in_=ident, pattern=[[1, P]],
                compare_op=mybir.AluOpType.is_equal, fill=0.0,
                base=0, channel_multiplier=-1,
            )

            def pe_broadcast_row(row, dst):
                """dst[P, T] = broadcast of row [1, T] to all partitions."""
                for c0 in range(0, T, TCH):
                    psb = psum.tile([P, TCH], F32, name="ps_main")
                    nc.tensor.matmul(
                        psb, ones_row, row[:, c0 : c0 + TCH],
                        start=True, stop=True,
                    )
                    nc.vector.tensor_copy(dst[:, c0 : c0 + TCH], psb)

            # ---------------- S0: global weight scales ----------------
            ph1 = contextlib.ExitStack()
            wraw_p = ph1.enter_context(tc.tile_pool(name="wraw", bufs=4))
            with tc.tile_pool(name="s0", bufs=8) as s0p, tc.tile_pool(
                name="s0t", bufs=8
            ) as s0t:
                acc3 = tiny.tile([P, 4], F32)
                nc.vector.memset(acc3, 0.0)
                # (src, acc col, rows, cols, piece width, load engine, abs engine)
                shard_specs = [
                    (wgs_d, 0, ff_sh, DM, min(2048, DM), nc.sync),
                    (wus_d, 1, ff_sh, DM, min(2048, DM), nc.scalar),
                    (wds_d, 2, dm_sh, FF, min(2048, FF), nc.scalar),
                ]
                idx = 0
                for src, col, rows, cols, pw, ldeng in shard_specs:
                    for r0 in range(0, rows, P):
                        pr = min(P, rows - r0)
                        for c0 in range(0, cols, pw):
                            t_in = s0p.tile([P, pw], F32, name="s0raw")
                            ldeng.dma_start(
                                t_in[:pr], src[r0 : r0 + pr, c0 : c0 + pw]
                            )
                            t_sum = s0t.tile([P, 1], F32, name="s0sum")
                            if idx % 2 == 0:
                                t_abs = s0p.tile([P, pw], F32, name="s0abs")
                                nc.scalar.activation(
                                    out=t_abs[:pr],
                                    in_=t_in[:pr],
                                    func=AFT.Abs,
                                    accum_out=t_sum[:pr],
                                )
                            else:
                                nc.vector.tensor_reduce(
                                    t_sum[:pr], t_in[:pr], axis=AXX, op=ADD,
                                    apply_absolute_value=True,
                                )
                            nc.vector.tensor_tensor(
                                out=acc3[:pr, col : col + 1],
                                in0=acc3[:pr, col : col + 1],
                                in1=t_sum[:pr],
                                op=ADD,
                            )
                            idx += 1
                # prefetch the first weight strips while the AllReduce
                # (queued behind on gpsimd) waits for the sums
                raw_pre = {}
                for b in range(2):
                    for wsrc, kk, ldeng in (
                        (wg_d, "g", nc.gpsimd), (wu_d, "u", nc.sync)
                    ):
                        rt = wraw_p.tile([P, DM], F32, name="wraw")
                        ldeng.dma_start(rt, wsrc[b * P : (b + 1) * P, :])
                        raw_pre[(b, kk)] = rt
                # cross-partition sum + 16B AllReduce for global sums
                ps_s = psum.tile([P, TCH], F32, name="ps_main")
                nc.tensor.matmul(
                    ps_s[:4, :1], acc3[:, :4], ones_col, start=True, stop=True
                )
                sb_s = s0t.tile([4, 1], F32, name="sb_s")
                nc.vector.tensor_copy(sb_s, ps_s[:4, :1])
                nc.sync.dma_start(cc_in[0, :4], sb_s[:, 0])
                nc.gpsimd.collective_compute(
                    "AllReduce",
                    ADD,
                    replica_groups=[list(range(ncores))],
                    ins=[cc_in[:].opt()],
                    outs=[cc_out[:].opt()],
                )

            # ---------------- phase 1: x-quant + gate/up -> h' [ff,tok] ----
            with ph1:
                xqt_p = ph1.enter_context(tc.tile_pool(name="xqt", bufs=1))
                xqt = xqt_p.tile([P, KD, T], BF16)  # XqT: [dm-part, k, token]
                acc_p = ph1.enter_context(tc.tile_pool(name="accp", bufs=1))
                acc_hi = acc_p.tile([P, T], F32)    # h' max partials (per tok)
                acc_lo = acc_p.tile([P, T], F32)    # h' min partials (per tok)
                nc.vector.memset(acc_hi, 0.0)
                nc.vector.memset(acc_lo, 0.0)
                bc_p = ph1.enter_context(tc.tile_pool(name="bc", bufs=4))

                # x quantization (per token-tile; loads on the sync queue —
                # the gpsimd queue is head-of-line blocked by the AllReduce
                # trigger until the S0 sums are ready)
                with tc.tile_pool(name="xw", bufs=4) as xw_p:
                    for m in range(MT):
                        xt = xw_p.tile([P, DM], F32, name="xt")
                        nc.sync.dma_start(xt, x_d[m * P : (m + 1) * P, :])
                        amax = xw_p.tile([P, 1], F32, name="amax")
                        nc.vector.tensor_reduce(
                            amax, xt, axis=AXX, op=MAX, apply_absolute_value=True
                        )
                        nc.vector.tensor_scalar(
                            out=sx_all[:, m : m + 1], in0=amax, scalar1=EPS,
                            scalar2=1.0 / 127.0, op0=MAX, op1=MULT,
                        )
                        nc.vector.reciprocal(
                            rx_all[:, m : m + 1], sx_all[:, m : m + 1]
                        )
                        xr = xw_p.tile([P, DM], F32, name="xr")
                        nc.vector.tensor_scalar(
                            out=xr, in0=xt, scalar1=rx_all[:, m : m + 1], scalar2=CR,
                            op0=MULT, op1=ADD,
                        )
                        xq = xw_p.tile([P, DM], BF16, name="xq")
                        nc.vector.tensor_scalar(
                            out=xq, in0=xr, scalar1=CR, scalar2=None,
                            op0=SUB, op1=BYPASS,
                        )
                        nc.sync.dma_start_transpose(
                            xqt[:, :, m * P : (m + 1) * P], xq
                        )
                        # stash sx column into the [1,T] row for broadcasting
                        nc.sync.dma_start(
                            sx_d[0, m * P : (m + 1) * P], sx_all[:, m : m + 1]
                        )

                # ---- S0 tail (gated on the AllReduce): emitted after
                # x-quant so the AR wait doesn't head-of-line block the
                # vector queue during startup
                sums_row = bc_p.tile([1, T], F32, name="bc_row")
                nc.sync.dma_start(sums_row[:, 0:4], cc_out[:])
                sw_row = bc_p.tile([1, T], F32, name="bc_row")
                nc.vector.tensor_scalar(
                    out=sw_row[:, 0:4], in0=sums_row[:, 0:4], scalar1=1.0 / NW,
                    scalar2=EPS, op0=MULT, op1=ADD,
                )
                nc.vector.reciprocal(sw_row[:, 4:8], sw_row[:, 0:4])
                row8 = bc_p.tile([1, T], F32, name="bc_row")
                nc.vector.tensor_scalar(
                    out=row8[:, 0:4], in0=sw_row[:, 4:8], scalar1=ALPHA,
                    scalar2=None, op0=MULT, op1=BYPASS,
                )
                nc.vector.tensor_copy(row8[:, 4:8], sw_row[:, 0:4])
                ps_b = psum.tile([P, TCH], F32, name="ps_main")
                nc.tensor.matmul(
                    ps_b[:, :8], ones_row, row8[:, 0:8], start=True, stop=True
                )
                nc.vector.tensor_copy(sb_scales, ps_b[:, :8])

                # per-token scale broadcast rows [P, T]
                sx_row = bc_p.tile([1, T], F32, name="bc_row")
                nc.sync.dma_start(sx_row, sx_d[:])
                sx_bc = bc_p.tile([P, T], F32, name="bc")
                pe_broadcast_row(sx_row, sx_bc)
                sxg_bc = bc_p.tile([P, T], F32, name="bc")
                nc.vector.tensor_scalar(
                    out=sxg_bc, in0=sx_bc, scalar1=sb_scales[:, 4:5],
                    scalar2=None, op0=MULT, op1=BYPASS,
                )
                sxu_bc = bc_p.tile([P, T], F32, name="bc")
                nc.vector.tensor_scalar(
                    out=sxu_bc, in0=sx_bc, scalar1=sb_scales[:, 5:6],
                    scalar2=None, op0=MULT, op1=BYPASS,
                )
                nc.vector.tensor_scalar(
                    out=sxu_all, in0=sx_all, scalar1=sb_scales[:, 5:6],
                    scalar2=None, op0=MULT, op1=BYPASS,
                )

                wtern_p = ph1.enter_context(tc.tile_pool(name="wtern", bufs=4))
                wchunk_p = ph1.enter_context(tc.tile_pool(name="wchunk", bufs=15))
                gt_p = ph1.enter_context(tc.tile_pool(name="gtp", bufs=4))
                hpr_p = ph1.enter_context(tc.tile_pool(name="hpr", bufs=4))

                def produce_chunk(wsrc, beta_col, b, teng, raw=None,
                                  ldeng=nc.gpsimd):
                    if raw is None:
                        raw = wraw_p.tile([P, DM], F32, name="wraw")
                        ldeng.dma_start(raw, wsrc[b * P : (b + 1) * P, :])
                    nc.scalar.activation(
                        out=raw, in_=raw, func=AFT.Tanh,
                        scale=sb_scales[:, beta_col : beta_col + 1],
                    )
                    tern = wtern_p.tile([P, DM], BF16, name="wtern")
                    nc.vector.tensor_scalar(
                        out=tern, in0=raw, scalar1=CR, scalar2=CR,
                        op0=ADD, op1=SUB,
                    )
                    ch = wchunk_p.tile([P, KD, P], BF16, name="wchunk")
                    teng.dma_start_transpose(ch, tern)
                    return ch

                for b in range(NFB):
                    chg = produce_chunk(
                        wg_d, 0, b, nc.sync, raw=raw_pre.get((b, "g")),
                        ldeng=nc.gpsimd,
                    )
                    chu = produce_chunk(
                        wu_d, 1, b, nc.sync, raw=raw_pre.get((b, "u")),
                        ldeng=nc.sync,
                    )
                    psg = [
                        psum.tile([P, TCH], F32, name="ps_main")
                        for _ in range(NTC)
                    ]
                    psu = [
                        psum.tile([P, TCH], F32, name="ps_main")
                        for _ in range(NTC)
                    ]
                    for k in range(KD):
                        st, sp = (k == 0), (k == KD - 1)
                        for t in range(NTC):
                            nc.tensor.matmul(
                                psg[t], chg[:, k, :],
                                xqt[:, k, t * TCH : (t + 1) * TCH],
                                start=st, stop=sp,
                            )
                        for t in range(NTC):
                            nc.tensor.matmul(
                                psu[t], chu[:, k, :],
                                xqt[:, k, t * TCH : (t + 1) * TCH],
                                start=st, stop=sp,
                            )
                    for t in range(NTC):
                        sl = slice(t * TCH, (t + 1) * TCH)
                        gt = gt_p.tile([P, TCH], F32, name="gt")
                        nc.vector.tensor_tensor(
                            out=gt, in0=psg[t], in1=sxg_bc[:, sl], op=MULT
                        )
                        nc.scalar.activation(out=gt, in_=gt, func=AFT.Sigmoid)
                        hp = hpr_p.tile([P, TCH], F32, name="hp")
                        nc.vector.tensor_tensor(
                            out=hp, in0=gt, in1=psu[t], op=MULT
                        )
                        nc.vector.tensor_tensor(
                            out=acc_hi[:, sl], in0=acc_hi[:, sl], in1=hp,
                            op=MAX,
                        )
                        nc.vector.tensor_tensor(
                            out=acc_lo[:, sl], in0=acc_lo[:, sl], in1=hp,
                            op=mybir.AluOpType.min,
                        )
                        nc.scalar.dma_start(hp_d[b, t], hp)

                # ---- h quantization scales ----
                # per-token absmax: PE-transpose the [ff-pos, tok] partials to
                # token-partition columns, reduce along free axis, then do the
                # scale math as columns [P, 1] per token tile (baseline form).
                with tc.tile_pool(name="hscl", bufs=8) as hs_p:
                    for m in range(MT):
                        msl = slice(m * P, (m + 1) * P)
                        pthi = psum.tile([P, P], F32, name="ps_main")
                        nc.tensor.transpose(pthi, acc_hi[:, msl], ident)
                        ptlo = psum.tile([P, P], F32, name="ps_main")
                        nc.tensor.transpose(ptlo, acc_lo[:, msl], ident)
                        chi = hs_p.tile([P, 1], F32, name="chi")
                        nc.vector.tensor_reduce(
                            chi, pthi, axis=AXX, op=MAX,
                            apply_absolute_value=True,
                        )
                        clo = hs_p.tile([P, 1], F32, name="clo")
                        nc.vector.tensor_reduce(
                            clo, ptlo, axis=AXX, op=MAX,
                            apply_absolute_value=True,
                        )
                        habs_c = hs_p.tile([P, 1], F32, name="habs_c")
                        nc.vector.tensor_tensor(
                            out=habs_c, in0=chi, in1=clo, op=MAX
                        )
                        sh_c = hs_p.tile([P, 1], F32, name="sh_c")
                        nc.vector.tensor_tensor(
                            out=sh_c, in0=habs_c, in1=sxu_all[:, m : m + 1],
                            op=MULT,
                        )
                        nc.vector.tensor_scalar(
                            out=sh_c, in0=sh_c, scalar1=EPS,
                            scalar2=1.0 / 127.0, op0=MAX, op1=MULT,
                        )
                        rec_c = hs_p.tile([P, 1], F32, name="rec_c")
                        nc.vector.reciprocal(rec_c, sh_c)
                        rph_c = hs_p.tile([P, 1], F32, name="rph_c")
                        nc.vector.tensor_tensor(
                            out=rph_c, in0=rec_c, in1=sxu_all[:, m : m + 1],
                            op=MULT,
                        )
                        shd_c = hs_p.tile([P, 1], F32, name="shd_c")
                        nc.vector.tensor_scalar(
                            out=shd_c, in0=sh_c, scalar1=sb_scales[:, 6:7],
                            scalar2=None, op0=MULT, op1=BYPASS,
                        )
                        nc.sync.dma_start(rph_d[0, msl], rph_c[:, 0:1])
                        nc.sync.dma_start(shd_d[0, msl], shd_c[:, 0:1])

                    rph_row = bc_p.tile([1, T], F32, name="bc_row")
                    nc.sync.dma_start(rph_row, rph_d[:])
                    rph_bc = bc2_p.tile([P, T], F32, name="bc2")
                    pe_broadcast_row(rph_row, rph_bc)
                    shd_row = bc_p.tile([1, T], F32, name="bc_row")
                    nc.sync.dma_start(shd_row, shd_d[:])
                    shd_bc = bc2_p.tile([P, T], F32, name="bc2")
                    pe_broadcast_row(shd_row, shd_bc)

            # ---------------- phase 3: quantize h' + down projection -------
            with contextlib.ExitStack() as ph3:
                hqt_p = ph3.enter_context(tc.tile_pool(name="hqt", bufs=1))
                hqt = hqt_p.tile([P, NFB, T], BF16)  # [ff-in-blk, ff-blk, tok]
                stage_p = ph3.enter_context(tc.tile_pool(name="stage", bufs=2))
                wdr_p = ph3.enter_context(tc.tile_pool(name="wdr", bufs=2))
                wdtern_p = ph3.enter_context(tc.tile_pool(name="wdtn", bufs=2))
                wdt_p = ph3.enter_context(tc.tile_pool(name="wdtg", bufs=2 * NW3))
                fin_p = ph3.enter_context(tc.tile_pool(name="finp", bufs=2))

                def quantize_block(b):
                    stage = stage_p.tile([P, T], F32, name="stage")
                    for t in range(NTC):
                        nc.scalar.dma_start(
                            stage[:, t * TCH : (t + 1) * TCH], hp_d[b, t]
                        )
                    stage2 = stage_p.tile([P, T], F32, name="stage")
                    nc.vector.tensor_tensor(
                        out=stage2, in0=stage, in1=rph_bc, op=MULT
                    )
                    nc.vector.tensor_scalar(
                        out=hqt[:, b, :], in0=stage2, scalar1=CR, scalar2=CR,
                        op0=ADD, op1=SUB,
                    )

                def produce_wd(md):
                    pieces = []
                    for w in range(NW3):
                        raw = wdr_p.tile([P, KW3], F32, name="wdraw")
                        nc.gpsimd.dma_start(
                            raw,
                            wd_d[md * P : (md + 1) * P, w * KW3 : (w + 1) * KW3],
                        )
                        nc.scalar.activation(
                            out=raw, in_=raw, func=AFT.Tanh,
                            scale=sb_scales[:, 2:3],
                        )
                        ternd = wdtern_p.tile([P, KW3], BF16, name="wdtern")
                        nc.vector.tensor_scalar(
                            out=ternd, in0=raw, scalar1=CR, scalar2=CR,
                            op0=ADD, op1=SUB,
                        )
                        wdtg = wdt_p.tile([P, KB3, P], BF16, name="wdtg")
                        nc.sync.dma_start_transpose(wdtg, ternd)
                        pieces.append(wdtg)
                    return pieces

                # Emit the first mds' weight pipelines ahead of the hqt fill
                # so their vector/scalar work isn't queued behind it; all
                # hqt writers must be emitted before any consuming matmul.
                NAHEAD = min(2, MD)
                wd_pieces = {md: produce_wd(md) for md in range(NAHEAD)}
                for b in range(NFB):
                    quantize_block(b)
                def emit_fin(md, pss):
                    for t in range(NTC):
                        sl = slice(t * TCH, (t + 1) * TCH)
                        ot = fin_p.tile([P, TCH], F32, name="ot")
                        nc.vector.tensor_tensor(
                            out=ot, in0=pss[t], in1=shd_bc[:, sl], op=MULT
                        )
                        nc.scalar.dma_start(
                            out_d[md * P : (md + 1) * P, sl], ot
                        )

                pending = []
                for md in range(MD):
                    pieces = wd_pieces.pop(md) if md in wd_pieces else produce_wd(md)
                    pss = [
                        psum.tile([P, TCH], F32, name="ps_main")
                        for _ in range(NTC)
                    ]
                    for k in range(NFB):
                        lhsT = pieces[k // KB3][:, k % KB3, :]
                        st, sp = (k == 0), (k == NFB - 1)
                        for t in range(NTC):
                            nc.tensor.matmul(
                                pss[t], lhsT,
                                hqt[:, k, t * TCH : (t + 1) * TCH],
                                start=st, stop=sp,
                            )
                    pending.append((md, pss))
                    if len(pending) >= 3:
                        emit_fin(*pending.pop(0))
                for it in pending:
                    emit_fin(*it)

    nc.compile()
    return nc


_CACHE = {}
TRACE = False  # set True (e.g. from test.py) to capture an NTFF profile
LAST_RESULTS = None


def _get_program(T, DM, FF, ncores, ff_sh, dm_sh):
    key = (T, DM, FF, ncores, ff_sh, dm_sh)
    if key not in _CACHE:
        _CACHE[key] = build_program(T, DM, FF, ncores, ff_sh, dm_sh)
    return _CACHE[key]


def kernel(x, w_gate, w_up, w_down):
    from concourse.bass_utils import run_bass_kernel_spmd

    x = np.asarray(x, dtype=np.float32)
    w_gate = np.ascontiguousarray(np.asarray(w_gate, dtype=np.float32))
    w_up = np.ascontiguousarray(np.asarray(w_up, dtype=np.float32))
    w_down = np.ascontiguousarray(np.asarray(w_down, dtype=np.float32))

    B, S, DM = x.shape
    FF = w_gate.shape[0]
    NCORES = 8
    NTOK = B * S
    T = NTOK // NCORES
    ff_sh = FF // NCORES
    dm_sh = DM // NCORES

    xf = np.ascontiguousarray(x.reshape(NTOK, DM))
    nc = _get_program(T, DM, FF, NCORES, ff_sh, dm_sh)

    in_maps = []
    for c in range(NCORES):
        in_maps.append(
            {
                "x": np.ascontiguousarray(xf[c * T : (c + 1) * T]),
                "wg": w_gate,
                "wu": w_up,
                "wd": w_down,
                "wg_sh": np.ascontiguousarray(w_gate[c * ff_sh : (c + 1) * ff_sh]),
                "wu_sh": np.ascontiguousarray(w_up[c * ff_sh : (c + 1) * ff_sh]),
                "wd_sh": np.ascontiguousarray(w_down[c * dm_sh : (c + 1) * dm_sh]),
            }
        )

    res = run_bass_kernel_spmd(
        nc, in_maps, core_ids=list(range(NCORES)), trace=TRACE
    )
    global LAST_RESULTS
    LAST_RESULTS = res
    out = np.empty((NTOK, DM), dtype=np.float32)
    for c in range(NCORES):
        out[c * T : (c + 1) * T] = res.results[c]["out_t"].T
    return out.reshape(B, S, DM)
